# revision 21
# baseline (speedup 1.0000x reference)
"""AttentionBlock (GroupNorm + single-head self-attention + residual) on 8 TRN2 cores.

Data-parallel over batch: core b handles x[b] (C=128, HW=4096).

Pipeline per core:
  1. GroupNorm with channels on partitions: per-channel bn_stats over HW, then a
     block-diagonal (1/16) matmul combines stats across each group's channels
     and broadcasts them back per channel; gamma/beta fold into per-partition
     scale/shift.
  2. Q^T,K^T [C,HW] in bf16 and V [HW,C] blocks in bf16 with a ones column
     appended (V_ext [*,129]).
  3. Attention in q-chunks of 1024:
       S^T tile [k=128, q=1024] = K_blk^T Q_chunk (bf16 matmuls, fp32 PSUM)
       P~ = exp(S^T/sqrt(C)) via ACT -> bf16 SBUF (no max subtraction: scores
       are ~N(0,1) here, far from fp32 overflow, and softmax normalization
       makes the result identical to jax.nn.softmax)
       PV+Z fused: out_nat[q128, 129] = sum_k P~^T[k,q].T @ V_ext[k,129]
       (column 128 = softmax denominator Z), accumulated over k blocks in one
       PSUM bank; normalize by 1/Z (per-partition scalar), PE-transpose back
       to channel-major.
     Chunk n+1's S/exp work is emitted before chunk n's PV so the scalar
     engine (exp is the critical resource) never starves.
  4. Output projection (fp32r), + bias + residual, DMA out.

fp32r = fp32-width storage the PE streams single-pass at 1.2 Gcol/s (vs 2-pass
for fp32); bf16 streams at 2.4 Gcol/s. The BIR verifier requires producers of
fp32r matmul operands to round on write, so fp32r tiles are written only by
compute ops (or an explicit rounding copy for DMA'd weights).
"""

import math
from contextlib import ExitStack

import numpy as np

import concourse.bacc as bacc
import concourse.bass as bass
import concourse.tile as tile
from concourse import mybir
from concourse.bass_utils import run_bass_kernel_spmd

B = 8
C = 128
HW = 4096
GROUPS = 8
GSIZE = C // GROUPS
EPS = 1e-5
NCORES = 8

QCHUNK = 1024          # q columns per attention chunk (2 PSUM banks for S^T)
NQC = HW // QCHUNK     # 4
KBLK = 128             # k rows per score tile (partition dim)
NKB = HW // KBLK       # 32
NSUB = QCHUNK // 128   # 8 q-subblocks per chunk for the natural-layout PV
VE = 132               # V_ext allocated width (129 used: 128 channels + ones)
F32 = mybir.dt.float32
BF16 = mybir.dt.bfloat16


def _emit(tc, d, ctx):
    nc = tc.nc
    scale = 1.0 / math.sqrt(C)
    F32R = mybir.dt.float32r

    consts = ctx.enter_context(tc.tile_pool(name="consts", bufs=1))
    bigp = ctx.enter_context(tc.tile_pool(name="bigp", bufs=1))
    ptp = ctx.enter_context(tc.tile_pool(name="ptp", bufs=66))
    outp = ctx.enter_context(tc.tile_pool(name="outp", bufs=3))
    smallp = ctx.enter_context(tc.tile_pool(name="smallp", bufs=2))
    ps_s = ctx.enter_context(tc.tile_pool(name="ps_s", bufs=2, space="PSUM"))
    ps_n = ctx.enter_context(tc.tile_pool(name="ps_n", bufs=2, space="PSUM"))
    ps_t = ctx.enter_context(tc.tile_pool(name="ps_t", bufs=2, space="PSUM"))

    # ---- constants into SBUF (packed: 2 DMAs instead of 12) ----
    # warm the exp ACT table set at t=0 so no table load hits the critical path
    warm_in = consts.tile([C, 1], F32)
    nc.vector.memset(warm_in, 0.0)
    warm_out = consts.tile([C, 1], F32)
    nc.scalar.activation(out=warm_out, in_=warm_in,
                         func=mybir.ActivationFunctionType.Exp)

    cmat = consts.tile([C, 7, C], F32)
    cvec = consts.tile([C, 5], F32)
    nc.sync.dma_start(out=cmat, in_=d["cmat"])
    nc.sync.dma_start(out=cvec, in_=d["cvec"])
    wqT, wkT, wvT, woT = (cmat[:, i, :] for i in range(4))
    gmat, bv_bc, ident = (cmat[:, i, :] for i in range(4, 7))
    bq, bk, bo = (cvec[:, i:i + 1] for i in range(3))
    gamma, beta = (cvec[:, i:i + 1] for i in range(3, 5))

    # x arrives in 512-column slices so bn_stats starts as early as possible.
    # Nothing input-dependent may be emitted on DVE before these bn_stats:
    # engine queues are in-order, and a stalled earlier op head-of-line
    # blocks the whole GroupNorm chain.
    x_sb = bigp.tile([C, HW], F32)
    stats = smallp.tile([C, HW // 512, 6], F32)
    for i in range(HW // 512):
        sl = slice(i * 512, (i + 1) * 512)
        nc.sync.dma_start(out=x_sb[:, sl], in_=d["x"][:, sl])
        nc.vector.bn_stats(out=stats[:, i, :], in_=x_sb[:, sl])
    mv = smallp.tile([C, 2], F32)
    nc.vector.bn_aggr(out=mv, in_=stats)

    # fp32r weight copies on GpSimd: keeps them off the DVE queue
    wqT_r = consts.tile([C, C], F32R)
    wkT_r = consts.tile([C, C], F32R)
    wvT_r = consts.tile([C, C], F32R)
    woT_r = consts.tile([C, C], F32R)
    for dst, srct in [(wqT_r, wqT), (wkT_r, wkT), (wvT_r, wvT), (woT_r, woT)]:
        nc.gpsimd.tensor_copy(out=dst, in_=srct)

    # stats2 = (mean, E[x^2]) per channel; gmat averages across each group.
    stats2 = smallp.tile([C, 2], F32)
    nc.vector.tensor_copy(out=stats2[:, 0:1], in_=mv[:, 0:1])
    nc.vector.tensor_mul(out=stats2[:, 1:2], in0=mv[:, 0:1], in1=mv[:, 0:1])
    nc.vector.tensor_add(out=stats2[:, 1:2], in0=stats2[:, 1:2], in1=mv[:, 1:2])
    ps_g = ps_s.tile([C, QCHUNK], F32, tag="ps_s", name="ps_g")
    nc.tensor.matmul(ps_g[:, 0:2], gmat, stats2)

    gstat = smallp.tile([C, 2], F32)
    nc.vector.tensor_copy(out=gstat, in_=ps_g[:, 0:2])
    varg = smallp.tile([C, 1], F32)
    nc.vector.tensor_mul(out=varg, in0=gstat[:, 0:1], in1=gstat[:, 0:1])
    nc.vector.tensor_sub(out=varg, in0=gstat[:, 1:2], in1=varg)
    # rstd = rsqrt(var + eps) via bit-trick seed + 3 Newton steps on DVE
    # (the ACT Sqrt lives in a different table set than Exp; using it would
    # put a ~1.3us ACT_TABLE_LOAD on the critical path twice)
    nc.vector.tensor_scalar_add(out=varg, in0=varg, scalar1=EPS)
    I32 = mybir.dt.int32
    magic = smallp.tile([C, 1], I32)
    nc.vector.memset(magic, 0x5F3759DF)
    rstd = smallp.tile([C, 1], F32)
    nc.vector.tensor_scalar(out=rstd.bitcast(I32), in0=varg.bitcast(I32),
                            scalar1=1, scalar2=None,
                            op0=mybir.AluOpType.arith_shift_right)
    nc.vector.tensor_sub(out=rstd.bitcast(I32), in0=magic, in1=rstd.bitcast(I32))
    nt = smallp.tile([C, 1], F32)
    for _ in range(3):
        nc.vector.tensor_mul(out=nt, in0=varg, in1=rstd)
        nc.vector.tensor_mul(out=nt, in0=nt, in1=rstd)
        nc.vector.tensor_scalar(out=nt, in0=nt, scalar1=-0.5, scalar2=1.5,
                                op0=mybir.AluOpType.mult,
                                op1=mybir.AluOpType.add)
        nc.vector.tensor_mul(out=rstd, in0=rstd, in1=nt)
    # xn = x*s1 + s2 with s1 = rstd*gamma, s2 = beta - mean*s1
    s1 = smallp.tile([C, 1], F32)
    s2 = smallp.tile([C, 1], F32)
    nc.vector.tensor_mul(out=s1, in0=rstd, in1=gamma)
    nc.vector.tensor_mul(out=s2, in0=gstat[:, 0:1], in1=s1)
    nc.vector.tensor_sub(out=s2, in0=beta, in1=s2)
    # minimal pre-attention work: xn/K/Q for the first two 512-slices only;
    # everything else streams in as gap-filler during chunk 0. Order matters:
    # engine queues are in-order, so nothing slow may sit ahead of the ops
    # that gate the first exp.
    q_bf = bigp.tile([C, HW], BF16)   # Q^T channel-major
    k_bf = bigp.tile([C, HW], BF16)   # K^T
    v_ext = bigp.tile([C, NKB, VE], BF16)  # V token-major + ones column
    nc.vector.memset(v_ext[:, :, 128:129], 1.0)
    xn_sb = bigp.tile([C, HW], mybir.dt.float32r, tag="xn_on", name="xn_sb")

    def xn_slice(h):
        sl = slice(h * 512, (h + 1) * 512)
        nc.vector.tensor_scalar(out=xn_sb[:, sl], in0=x_sb[:, sl],
                                scalar1=s1, scalar2=s2,
                                op0=mybir.AluOpType.mult, op1=mybir.AluOpType.add)

    def k_slice(h, pool, tag):
        sl = slice(h * 512, (h + 1) * 512)
        ps_k = pool.tile([C, 512], F32, tag=tag, name="ps_k")
        nc.tensor.matmul(ps_k[:, 0:512], wkT_r, xn_sb[:, sl])
        nc.vector.tensor_scalar_add(out=k_bf[:, sl], in0=ps_k[:, 0:512], scalar1=bk)

    def q_slice(h, pool, tag):
        sl = slice(h * 512, (h + 1) * 512)
        ps_q = pool.tile([C, 512], F32, tag=tag, name="ps_q")
        nc.tensor.matmul(ps_q[:, 0:512], wqT_r, xn_sb[:, sl])
        nc.vector.tensor_scalar_add(out=q_bf[:, sl], in0=ps_q[:, 0:512], scalar1=bq)

    xn_slice(0)
    xn_slice(1)
    k_slice(0, ps_s, "ps_s")
    k_slice(1, ps_s, "ps_s")
    q_slice(0, ps_s, "ps_s")
    q_slice(1, ps_s, "ps_s")

    def head_work():
        """Remaining xn/K/Q slices and all V blocks, emitted in unit-sized
        slices as PE gap-filler inside chunk 0's S/exp stream. Uses ps_t/ps_n
        pools so it never competes for the S-tile PSUM slots."""
        for h in range(2, HW // 512):
            xn_slice(h)
            k_slice(h, ps_t, "ps_t")
            q_slice(h, ps_t, "ps_t")
            yield
        for kb0 in range(0, NKB, 4):
            for kb in range(kb0, kb0 + 4):
                ps_v = ps_n.tile([C, VE], F32, tag="ps_n", name="ps_v")
                nc.tensor.matmul(ps_v[:, 0:KBLK],
                                 xn_sb[:, kb * KBLK:(kb + 1) * KBLK], wvT_r)
                nc.vector.tensor_add(out=v_ext[:, kb, 0:128],
                                     in0=ps_v[:, 0:KBLK], in1=bv_bc)
            yield

    # ---- attention ----
    on_sb = bigp.tile([C, HW], mybir.dt.float32r, tag="xn_on", name="on_sb")
    PV_SLICE = 8  # PV matmuls per S/exp unit (~0.7us of PE gap-filler)

    def proj_out(h):
        sl = slice(h * 512, (h + 1) * 512)
        ps_f = ps_t.tile([C, 512], F32, tag="ps_t", name="ps_f")
        nc.tensor.matmul(ps_f[:, 0:512], woT_r, on_sb[:, sl])
        ot = outp.tile([C, 512], F32, name="ot")
        nc.vector.scalar_tensor_tensor(out=ot, in0=ps_f[:, 0:512], scalar=bo,
                                       in1=x_sb[:, sl],
                                       op0=mybir.AluOpType.add,
                                       op1=mybir.AluOpType.add)
        nc.sync.dma_start(out=d["out"][:, sl], in_=ot)

    def tail_work(qc, pts):
        """PV + normalize + transpose + output projection for chunk qc,
        yielded in ~unit-sized slices to pace evenly against S/exp units."""
        for sub in range(NSUB):
            qpos = qc * QCHUNK + sub * KBLK
            pnat = ps_n.tile([C, VE], F32, tag="ps_n", name="pnat")
            for kb0 in range(0, NKB, PV_SLICE):
                for kb in range(kb0, kb0 + PV_SLICE):
                    nc.tensor.matmul(pnat[:, 0:129],
                                     pts[kb][:, sub * KBLK:(sub + 1) * KBLK],
                                     v_ext[:, kb, 0:129],
                                     start=(kb == 0), stop=(kb == NKB - 1))
                if kb0 + PV_SLICE < NKB:
                    yield
            rz = smallp.tile([C, 1], F32, name="rz")
            nc.vector.reciprocal(out=rz, in_=pnat[:, 128:129])
            onat = outp.tile([C, KBLK], F32, name="onat")
            nc.vector.tensor_scalar_mul(out=onat, in0=pnat[:, 0:128], scalar1=rz)
            ptr = ps_t.tile([C, KBLK], F32, tag="ps_t", name="ptr")
            nc.tensor.transpose(ptr, onat, ident)
            nc.vector.tensor_copy(out=on_sb[:, qpos:qpos + KBLK], in_=ptr)
            if sub % 4 == 3:
                proj_out(qc * 2 + sub // 4)
            yield

    filler = head_work()
    for qc in range(NQC):
        q0 = qc * QCHUNK
        pts = []
        for kb in range(NKB):
            ksl = slice(kb * KBLK, (kb + 1) * KBLK)
            ps_sc = ps_s.tile([C, QCHUNK], F32, tag="ps_s", name="ps_sc")
            nc.tensor.matmul(ps_sc[:, 0:512], k_bf[:, ksl], q_bf[:, q0:q0 + 512])
            nc.tensor.matmul(ps_sc[:, 512:1024], k_bf[:, ksl],
                             q_bf[:, q0 + 512:q0 + 1024])
            pt = ptp.tile([C, QCHUNK], BF16, name="pt")
            nc.scalar.activation(out=pt, in_=ps_sc,
                                 func=mybir.ActivationFunctionType.Exp,
                                 scale=scale)
            pts.append(pt)
            next(filler, None)
        filler = tail_work(qc, pts)
    for _ in filler:
        pass


_CACHE = {}


def _build():
    if "nc" in _CACHE:
        return _CACHE["nc"]
    nc = bacc.Bacc("TRN2", target_bir_lowering=False, debug=False,
                   num_devices=NCORES)
    d = {}
    d["x"] = nc.dram_tensor("x", [C, HW], F32, kind="ExternalInput").ap()
    d["cmat"] = nc.dram_tensor("cmat", [C, 7, C], F32, kind="ExternalInput").ap()
    d["cvec"] = nc.dram_tensor("cvec", [C, 5], F32, kind="ExternalInput").ap()
    d["out"] = nc.dram_tensor("out", [C, HW], F32, kind="ExternalOutput").ap()
    with tile.TileContext(nc) as tc:
        with ExitStack() as ctx:
            _emit(tc, d, ctx)
    nc.compile()
    _CACHE["nc"] = nc
    return nc


def make_in_maps(x, gamma, beta, wq, bq, wk, bk, wv, bv, wo, bo):
    f = np.float32
    gm = np.zeros((C, C), f)
    for g in range(GROUPS):
        gm[g * GSIZE:(g + 1) * GSIZE, g * GSIZE:(g + 1) * GSIZE] = 1.0 / GSIZE
    cmat = np.stack([
        np.asarray(wq, f).T, np.asarray(wk, f).T, np.asarray(wv, f).T,
        np.asarray(wo, f).T, gm,
        np.tile(np.asarray(bv, f).reshape(1, C), (C, 1)),
        np.eye(C, dtype=f),
    ], axis=1)  # [C, 7, C]
    cvec = np.stack([np.asarray(v, f).reshape(C) for v in
                     (bq, bk, bo, gamma, beta)], axis=1)  # [C, 5]
    common = {
        "cmat": np.ascontiguousarray(cmat),
        "cvec": np.ascontiguousarray(cvec),
    }
    xf = np.asarray(x, f).reshape(B, C, HW)
    return [dict(common, x=np.ascontiguousarray(xf[b])) for b in range(B)]


def kernel(x, gamma, beta, wq, bq, wk, bk, wv, bv, wo, bo, **run_kwargs):
    nc = _build()
    in_maps = make_in_maps(x, gamma, beta, wq, bq, wk, bk, wv, bv, wo, bo)
    res = run_bass_kernel_spmd(nc, in_maps, core_ids=list(range(NCORES)),
                               **run_kwargs)
    out = np.stack([res.results[b]["out"] for b in range(B)])
    _CACHE["last_results"] = res
    return out.reshape(B, C, 64, 64).astype(np.float32)


# revision 23
# speedup vs baseline: 1.0180x; 1.0180x over previous
"""AttentionBlock (GroupNorm + single-head self-attention + residual) on 8 TRN2 cores.

Data-parallel over batch: core b handles x[b] (C=128, HW=4096).

Pipeline per core:
  1. GroupNorm with channels on partitions: per-channel bn_stats over HW, then a
     block-diagonal (1/16) matmul combines stats across each group's channels
     and broadcasts them back per channel; gamma/beta fold into per-partition
     scale/shift.
  2. Q^T,K^T [C,HW] in bf16 and V [HW,C] blocks in bf16 with a ones column
     appended (V_ext [*,129]).
  3. Attention in q-chunks of 1024:
       S^T tile [k=128, q=1024] = K_blk^T Q_chunk (bf16 matmuls, fp32 PSUM)
       P~ = exp(S^T/sqrt(C)) via ACT -> bf16 SBUF (no max subtraction: scores
       are ~N(0,1) here, far from fp32 overflow, and softmax normalization
       makes the result identical to jax.nn.softmax)
       PV+Z fused: out_nat[q128, 129] = sum_k P~^T[k,q].T @ V_ext[k,129]
       (column 128 = softmax denominator Z), accumulated over k blocks in one
       PSUM bank; normalize by 1/Z (per-partition scalar), PE-transpose back
       to channel-major.
     Chunk n+1's S/exp work is emitted before chunk n's PV so the scalar
     engine (exp is the critical resource) never starves.
  4. Output projection (fp32r), + bias + residual, DMA out.

fp32r = fp32-width storage the PE streams single-pass at 1.2 Gcol/s (vs 2-pass
for fp32); bf16 streams at 2.4 Gcol/s. The BIR verifier requires producers of
fp32r matmul operands to round on write, so fp32r tiles are written only by
compute ops (or an explicit rounding copy for DMA'd weights).
"""

import math
from contextlib import ExitStack

import numpy as np

import concourse.bacc as bacc
import concourse.bass as bass
import concourse.tile as tile
from concourse import mybir
from concourse.bass_utils import run_bass_kernel_spmd

B = 8
C = 128
HW = 4096
GROUPS = 8
GSIZE = C // GROUPS
EPS = 1e-5
NCORES = 8

QCHUNK = 1024          # q columns per attention chunk (2 PSUM banks for S^T)
NQC = HW // QCHUNK     # 4
KBLK = 128             # k rows per score tile (partition dim)
NKB = HW // KBLK       # 32
NSUB = QCHUNK // 128   # 8 q-subblocks per chunk for the natural-layout PV
VE = 132               # V_ext allocated width (129 used: 128 channels + ones)
F32 = mybir.dt.float32
BF16 = mybir.dt.bfloat16


def _emit(tc, d, ctx):
    nc = tc.nc
    scale = 1.0 / math.sqrt(C)
    F32R = mybir.dt.float32r

    consts = ctx.enter_context(tc.tile_pool(name="consts", bufs=1))
    bigp = ctx.enter_context(tc.tile_pool(name="bigp", bufs=1))
    ptp = ctx.enter_context(tc.tile_pool(name="ptp", bufs=66))
    outp = ctx.enter_context(tc.tile_pool(name="outp", bufs=3))
    smallp = ctx.enter_context(tc.tile_pool(name="smallp", bufs=2))
    ps_s = ctx.enter_context(tc.tile_pool(name="ps_s", bufs=2, space="PSUM"))
    ps_n = ctx.enter_context(tc.tile_pool(name="ps_n", bufs=2, space="PSUM"))
    ps_t = ctx.enter_context(tc.tile_pool(name="ps_t", bufs=2, space="PSUM"))

    # ---- constants into SBUF (packed: 2 DMAs instead of 12) ----
    # warm the exp ACT table set at t=0 so no table load hits the critical path
    warm_in = consts.tile([C, 1], F32)
    nc.vector.memset(warm_in, 0.0)
    warm_out = consts.tile([C, 1], F32)
    nc.scalar.activation(out=warm_out, in_=warm_in,
                         func=mybir.ActivationFunctionType.Exp)

    cmat = consts.tile([C, 7, C], F32)
    cvec = consts.tile([C, 5], F32)
    nc.scalar.dma_start(out=cmat, in_=d["cmat"])
    nc.scalar.dma_start(out=cvec, in_=d["cvec"])
    wqT, wkT, wvT, woT = (cmat[:, i, :] for i in range(4))
    gmat, bv_bc, ident = (cmat[:, i, :] for i in range(4, 7))
    bq, bk, bo = (cvec[:, i:i + 1] for i in range(3))
    gamma, beta = (cvec[:, i:i + 1] for i in range(3, 5))

    # x arrives in 512-column slices so bn_stats starts as early as possible.
    # Nothing input-dependent may be emitted on DVE before these bn_stats:
    # engine queues are in-order, and a stalled earlier op head-of-line
    # blocks the whole GroupNorm chain.
    x_sb = bigp.tile([C, HW], F32)
    stats = smallp.tile([C, HW // 512, 6], F32)
    for i in range(HW // 512):
        sl = slice(i * 512, (i + 1) * 512)
        eng = nc.sync if i % 2 == 0 else nc.scalar
        eng.dma_start(out=x_sb[:, sl], in_=d["x"][:, sl])
        nc.vector.bn_stats(out=stats[:, i, :], in_=x_sb[:, sl])
    mv = smallp.tile([C, 2], F32)
    nc.vector.bn_aggr(out=mv, in_=stats)

    # fp32r weight copies on GpSimd: keeps them off the DVE queue
    wqT_r = consts.tile([C, C], F32R)
    wkT_r = consts.tile([C, C], F32R)
    wvT_r = consts.tile([C, C], F32R)
    woT_r = consts.tile([C, C], F32R)
    for dst, srct in [(wqT_r, wqT), (wkT_r, wkT), (wvT_r, wvT), (woT_r, woT)]:
        nc.gpsimd.tensor_copy(out=dst, in_=srct)

    # stats2 = (mean, E[x^2]) per channel; gmat averages across each group.
    stats2 = smallp.tile([C, 2], F32)
    nc.vector.tensor_copy(out=stats2[:, 0:1], in_=mv[:, 0:1])
    nc.vector.scalar_tensor_tensor(out=stats2[:, 1:2], in0=mv[:, 0:1],
                                   scalar=mv[:, 0:1], in1=mv[:, 1:2],
                                   op0=mybir.AluOpType.mult,
                                   op1=mybir.AluOpType.add)
    ps_g = ps_s.tile([C, QCHUNK], F32, tag="ps_s", name="ps_g")
    nc.tensor.matmul(ps_g[:, 0:2], gmat, stats2)

    gstat = smallp.tile([C, 2], F32)
    nc.vector.tensor_copy(out=gstat, in_=ps_g[:, 0:2])
    varg = smallp.tile([C, 1], F32)
    nc.vector.tensor_mul(out=varg, in0=gstat[:, 0:1], in1=gstat[:, 0:1])
    nc.vector.tensor_sub(out=varg, in0=gstat[:, 1:2], in1=varg)
    # rstd = rsqrt(var + eps) via bit-trick seed + 3 Newton steps on DVE
    # (the ACT Sqrt lives in a different table set than Exp; using it would
    # put a ~1.3us ACT_TABLE_LOAD on the critical path twice)
    nc.vector.tensor_scalar_add(out=varg, in0=varg, scalar1=EPS)
    I32 = mybir.dt.int32
    magic = smallp.tile([C, 1], I32)
    nc.vector.memset(magic, 0x5F3759DF)
    rstd = smallp.tile([C, 1], F32)
    nc.vector.tensor_scalar(out=rstd.bitcast(I32), in0=varg.bitcast(I32),
                            scalar1=1, scalar2=None,
                            op0=mybir.AluOpType.arith_shift_right)
    nc.vector.tensor_sub(out=rstd.bitcast(I32), in0=magic, in1=rstd.bitcast(I32))
    nt = smallp.tile([C, 1], F32)
    for _ in range(2):
        nc.vector.tensor_mul(out=nt, in0=varg, in1=rstd)
        nc.vector.tensor_mul(out=nt, in0=nt, in1=rstd)
        nc.vector.tensor_scalar(out=nt, in0=nt, scalar1=-0.5, scalar2=1.5,
                                op0=mybir.AluOpType.mult,
                                op1=mybir.AluOpType.add)
        nc.vector.tensor_mul(out=rstd, in0=rstd, in1=nt)
    # xn = x*s1 + s2 with s1 = rstd*gamma, s2 = beta - mean*s1
    s1 = smallp.tile([C, 1], F32)
    s2 = smallp.tile([C, 1], F32)
    nc.vector.tensor_mul(out=s1, in0=rstd, in1=gamma)
    nc.vector.tensor_mul(out=s2, in0=gstat[:, 0:1], in1=s1)
    nc.vector.tensor_sub(out=s2, in0=beta, in1=s2)
    # minimal pre-attention work: xn/K/Q for the first two 512-slices only;
    # everything else streams in as gap-filler during chunk 0. Order matters:
    # engine queues are in-order, so nothing slow may sit ahead of the ops
    # that gate the first exp.
    q_bf = bigp.tile([C, HW], BF16)   # Q^T channel-major
    k_bf = bigp.tile([C, HW], BF16)   # K^T
    v_ext = bigp.tile([C, NKB, VE], BF16)  # V token-major + ones column
    nc.vector.memset(v_ext[:, :, 128:129], 1.0)
    xn_sb = bigp.tile([C, HW], mybir.dt.float32r, tag="xn_on", name="xn_sb")

    def xn_slice(h):
        sl = slice(h * 512, (h + 1) * 512)
        nc.vector.tensor_scalar(out=xn_sb[:, sl], in0=x_sb[:, sl],
                                scalar1=s1, scalar2=s2,
                                op0=mybir.AluOpType.mult, op1=mybir.AluOpType.add)

    def k_slice(h, pool, tag):
        sl = slice(h * 512, (h + 1) * 512)
        ps_k = pool.tile([C, 512], F32, tag=tag, name="ps_k")
        nc.tensor.matmul(ps_k[:, 0:512], wkT_r, xn_sb[:, sl])
        nc.vector.tensor_scalar_add(out=k_bf[:, sl], in0=ps_k[:, 0:512], scalar1=bk)

    def q_slice(h, pool, tag):
        sl = slice(h * 512, (h + 1) * 512)
        ps_q = pool.tile([C, 512], F32, tag=tag, name="ps_q")
        nc.tensor.matmul(ps_q[:, 0:512], wqT_r, xn_sb[:, sl])
        nc.vector.tensor_scalar_add(out=q_bf[:, sl], in0=ps_q[:, 0:512], scalar1=bq)

    xn_slice(0)
    xn_slice(1)
    k_slice(0, ps_s, "ps_s")
    k_slice(1, ps_s, "ps_s")
    q_slice(0, ps_t, "ps_t")
    q_slice(1, ps_t, "ps_t")

    def head_work():
        """Remaining xn/K/Q slices and all V blocks, emitted in unit-sized
        slices as PE gap-filler inside chunk 0's S/exp stream. Uses ps_t/ps_n
        pools so it never competes for the S-tile PSUM slots."""
        for h in range(2, HW // 512):
            xn_slice(h)
            k_slice(h, ps_t, "ps_t")
            q_slice(h, ps_t, "ps_t")
            yield
        for kb0 in range(0, NKB, 4):
            for kb in range(kb0, kb0 + 4):
                ps_v = ps_n.tile([C, VE], F32, tag="ps_n", name="ps_v")
                nc.tensor.matmul(ps_v[:, 0:KBLK],
                                 xn_sb[:, kb * KBLK:(kb + 1) * KBLK], wvT_r)
                nc.vector.tensor_add(out=v_ext[:, kb, 0:128],
                                     in0=ps_v[:, 0:KBLK], in1=bv_bc)
            yield

    # ---- attention ----
    on_sb = bigp.tile([C, HW], mybir.dt.float32r, tag="xn_on", name="on_sb")
    PV_SLICE = 8  # PV matmuls per S/exp unit (~0.7us of PE gap-filler)

    def proj_out(h):
        sl = slice(h * 512, (h + 1) * 512)
        ps_f = ps_t.tile([C, 512], F32, tag="ps_t", name="ps_f")
        nc.tensor.matmul(ps_f[:, 0:512], woT_r, on_sb[:, sl])
        ot = outp.tile([C, 512], F32, name="ot")
        nc.vector.scalar_tensor_tensor(out=ot, in0=ps_f[:, 0:512], scalar=bo,
                                       in1=x_sb[:, sl],
                                       op0=mybir.AluOpType.add,
                                       op1=mybir.AluOpType.add)
        nc.sync.dma_start(out=d["out"][:, sl], in_=ot)

    def tail_work(qc, pts):
        """PV + normalize + transpose + output projection for chunk qc,
        yielded in ~unit-sized slices to pace evenly against S/exp units."""
        for sub in range(NSUB):
            qpos = qc * QCHUNK + sub * KBLK
            pnat = ps_n.tile([C, VE], F32, tag="ps_n", name="pnat")
            for kb0 in range(0, NKB, PV_SLICE):
                for kb in range(kb0, kb0 + PV_SLICE):
                    nc.tensor.matmul(pnat[:, 0:129],
                                     pts[kb][:, sub * KBLK:(sub + 1) * KBLK],
                                     v_ext[:, kb, 0:129],
                                     start=(kb == 0), stop=(kb == NKB - 1))
                if kb0 + PV_SLICE < NKB:
                    yield
            rz = smallp.tile([C, 1], F32, name="rz")
            nc.vector.reciprocal(out=rz, in_=pnat[:, 128:129])
            onat = outp.tile([C, KBLK], F32, name="onat")
            nc.vector.tensor_scalar_mul(out=onat, in0=pnat[:, 0:128], scalar1=rz)
            ptr = ps_t.tile([C, KBLK], F32, tag="ps_t", name="ptr")
            nc.tensor.transpose(ptr, onat, ident)
            nc.vector.tensor_copy(out=on_sb[:, qpos:qpos + KBLK], in_=ptr)
            if sub % 4 == 3:
                proj_out(qc * 2 + sub // 4)
            yield

    filler = head_work()
    for qc in range(NQC):
        q0 = qc * QCHUNK
        pts = []
        for kb in range(NKB):
            ksl = slice(kb * KBLK, (kb + 1) * KBLK)
            ps_sc = ps_s.tile([C, QCHUNK], F32, tag="ps_s", name="ps_sc")
            nc.tensor.matmul(ps_sc[:, 0:512], k_bf[:, ksl], q_bf[:, q0:q0 + 512])
            nc.tensor.matmul(ps_sc[:, 512:1024], k_bf[:, ksl],
                             q_bf[:, q0 + 512:q0 + 1024])
            pt = ptp.tile([C, QCHUNK], BF16, name="pt")
            nc.scalar.activation(out=pt, in_=ps_sc,
                                 func=mybir.ActivationFunctionType.Exp,
                                 scale=scale)
            pts.append(pt)
            next(filler, None)
        filler = tail_work(qc, pts)
    for _ in filler:
        pass


_CACHE = {}


def _build():
    if "nc" in _CACHE:
        return _CACHE["nc"]
    nc = bacc.Bacc("TRN2", target_bir_lowering=False, debug=False,
                   num_devices=NCORES)
    d = {}
    d["x"] = nc.dram_tensor("x", [C, HW], F32, kind="ExternalInput").ap()
    d["cmat"] = nc.dram_tensor("cmat", [C, 7, C], F32, kind="ExternalInput").ap()
    d["cvec"] = nc.dram_tensor("cvec", [C, 5], F32, kind="ExternalInput").ap()
    d["out"] = nc.dram_tensor("out", [C, HW], F32, kind="ExternalOutput").ap()
    with tile.TileContext(nc) as tc:
        with ExitStack() as ctx:
            _emit(tc, d, ctx)
    nc.compile()
    _CACHE["nc"] = nc
    return nc


def make_in_maps(x, gamma, beta, wq, bq, wk, bk, wv, bv, wo, bo):
    f = np.float32
    gm = np.zeros((C, C), f)
    for g in range(GROUPS):
        gm[g * GSIZE:(g + 1) * GSIZE, g * GSIZE:(g + 1) * GSIZE] = 1.0 / GSIZE
    cmat = np.stack([
        np.asarray(wq, f).T, np.asarray(wk, f).T, np.asarray(wv, f).T,
        np.asarray(wo, f).T, gm,
        np.tile(np.asarray(bv, f).reshape(1, C), (C, 1)),
        np.eye(C, dtype=f),
    ], axis=1)  # [C, 7, C]
    cvec = np.stack([np.asarray(v, f).reshape(C) for v in
                     (bq, bk, bo, gamma, beta)], axis=1)  # [C, 5]
    common = {
        "cmat": np.ascontiguousarray(cmat),
        "cvec": np.ascontiguousarray(cvec),
    }
    xf = np.asarray(x, f).reshape(B, C, HW)
    return [dict(common, x=np.ascontiguousarray(xf[b])) for b in range(B)]


def kernel(x, gamma, beta, wq, bq, wk, bk, wv, bv, wo, bo, **run_kwargs):
    nc = _build()
    in_maps = make_in_maps(x, gamma, beta, wq, bq, wk, bk, wv, bv, wo, bo)
    res = run_bass_kernel_spmd(nc, in_maps, core_ids=list(range(NCORES)),
                               **run_kwargs)
    out = np.stack([res.results[b]["out"] for b in range(B)])
    _CACHE["last_results"] = res
    return out.reshape(B, C, 64, 64).astype(np.float32)


# revision 24
# speedup vs baseline: 1.1971x; 1.1760x over previous
"""AttentionBlock (GroupNorm + single-head self-attention + residual) on 8 TRN2 cores.

Data-parallel over batch: core b handles x[b] (C=128, HW=4096).

Pipeline per core:
  1. GroupNorm with channels on partitions: per-channel bn_stats over HW, then a
     block-diagonal (1/16) matmul combines stats across each group's channels
     and broadcasts them back per channel; gamma/beta fold into per-partition
     scale/shift.
  2. Q^T,K^T [C,HW] in bf16 and V [HW,C] blocks in bf16 with a ones column
     appended (V_ext [*,129]).
  3. Attention in q-chunks of 1024:
       S^T tile [k=128, q=1024] = K_blk^T Q_chunk (bf16 matmuls, fp32 PSUM)
       P~ = exp(S^T/sqrt(C)) via ACT -> bf16 SBUF (no max subtraction: scores
       are ~N(0,1) here, far from fp32 overflow, and softmax normalization
       makes the result identical to jax.nn.softmax)
       PV+Z fused: out_nat[q128, 129] = sum_k P~^T[k,q].T @ V_ext[k,129]
       (column 128 = softmax denominator Z), accumulated over k blocks in one
       PSUM bank; normalize by 1/Z (per-partition scalar), PE-transpose back
       to channel-major.
     Chunk n+1's S/exp work is emitted before chunk n's PV so the scalar
     engine (exp is the critical resource) never starves.
  4. Output projection (fp32r), + bias + residual, DMA out.

fp32r = fp32-width storage the PE streams single-pass at 1.2 Gcol/s (vs 2-pass
for fp32); bf16 streams at 2.4 Gcol/s. The BIR verifier requires producers of
fp32r matmul operands to round on write, so fp32r tiles are written only by
compute ops (or an explicit rounding copy for DMA'd weights).
"""

import math
from contextlib import ExitStack

import numpy as np

import concourse.bacc as bacc
import concourse.bass as bass
import concourse.tile as tile
from concourse import mybir
from concourse.bass_utils import run_bass_kernel_spmd

B = 8
C = 128
HW = 4096
GROUPS = 8
GSIZE = C // GROUPS
EPS = 1e-5
NCORES = 8

QCHUNK = 1024          # q columns per attention chunk (2 PSUM banks for S^T)
NQC = HW // QCHUNK     # 4
KBLK = 128             # k rows per score tile (partition dim)
NKB = HW // KBLK       # 32
NSUB = QCHUNK // 128   # 8 q-subblocks per chunk for the natural-layout PV
VE = 132               # V_ext allocated width (129 used: 128 channels + ones)
F32 = mybir.dt.float32
BF16 = mybir.dt.bfloat16


def _emit(tc, d, ctx):
    nc = tc.nc
    scale = 1.0 / math.sqrt(C)
    F32R = mybir.dt.float32r

    consts = ctx.enter_context(tc.tile_pool(name="consts", bufs=1))
    bigp = ctx.enter_context(tc.tile_pool(name="bigp", bufs=1))
    ptp = ctx.enter_context(tc.tile_pool(name="ptp", bufs=66))
    outp = ctx.enter_context(tc.tile_pool(name="outp", bufs=3))
    smallp = ctx.enter_context(tc.tile_pool(name="smallp", bufs=2))
    ps_s = ctx.enter_context(tc.tile_pool(name="ps_s", bufs=2, space="PSUM"))
    ps_n = ctx.enter_context(tc.tile_pool(name="ps_n", bufs=2, space="PSUM"))
    ps_t = ctx.enter_context(tc.tile_pool(name="ps_t", bufs=2, space="PSUM"))

    # ---- constants into SBUF (packed: 2 DMAs instead of 12) ----
    # warm the exp ACT table set at t=0 so no table load hits the critical path
    warm_in = consts.tile([C, 1], F32)
    nc.vector.memset(warm_in, 0.0)
    warm_out = consts.tile([C, 1], F32)
    nc.scalar.activation(out=warm_out, in_=warm_in,
                         func=mybir.ActivationFunctionType.Exp)

    cmat = consts.tile([C, 7, C], F32)
    cvec = consts.tile([C, 5], F32)
    nc.scalar.dma_start(out=cmat, in_=d["cmat"])
    nc.scalar.dma_start(out=cvec, in_=d["cvec"])
    wqT, wkT, wvT, woT = (cmat[:, i, :] for i in range(4))
    gmat, bv_bc, ident = (cmat[:, i, :] for i in range(4, 7))
    bq, bk, bo = (cvec[:, i:i + 1] for i in range(3))
    gamma, beta = (cvec[:, i:i + 1] for i in range(3, 5))

    # x arrives in 512-column slices so bn_stats starts as early as possible.
    # Nothing input-dependent may be emitted on DVE before these bn_stats:
    # engine queues are in-order, and a stalled earlier op head-of-line
    # blocks the whole GroupNorm chain.
    x_sb = bigp.tile([C, HW], F32)
    stats = smallp.tile([C, HW // 512, 6], F32)
    for i in range(HW // 512):
        sl = slice(i * 512, (i + 1) * 512)
        eng = nc.sync if i % 2 == 0 else nc.scalar
        eng.dma_start(out=x_sb[:, sl], in_=d["x"][:, sl])
        nc.vector.bn_stats(out=stats[:, i, :], in_=x_sb[:, sl])
    mv = smallp.tile([C, 2], F32)
    nc.vector.bn_aggr(out=mv, in_=stats)

    # fp32r weight copies on GpSimd: keeps them off the DVE queue
    wqT_r = consts.tile([C, C], F32R)
    wkT_r = consts.tile([C, C], F32R)
    wvT_r = consts.tile([C, C], F32R)
    woT_r = consts.tile([C, C], F32R)
    for dst, srct in [(wqT_r, wqT), (wkT_r, wkT), (wvT_r, wvT), (woT_r, woT)]:
        nc.gpsimd.tensor_copy(out=dst, in_=srct)

    # stats2 = (mean, E[x^2]) per channel; gmat averages across each group.
    stats2 = smallp.tile([C, 2], F32)
    nc.vector.tensor_copy(out=stats2[:, 0:1], in_=mv[:, 0:1])
    nc.vector.scalar_tensor_tensor(out=stats2[:, 1:2], in0=mv[:, 0:1],
                                   scalar=mv[:, 0:1], in1=mv[:, 1:2],
                                   op0=mybir.AluOpType.mult,
                                   op1=mybir.AluOpType.add)
    ps_g = ps_s.tile([C, QCHUNK], F32, tag="ps_s", name="ps_g")
    nc.tensor.matmul(ps_g[:, 0:2], gmat, stats2)

    gstat = smallp.tile([C, 2], F32)
    nc.vector.tensor_copy(out=gstat, in_=ps_g[:, 0:2])
    varg = smallp.tile([C, 1], F32)
    nc.vector.tensor_mul(out=varg, in0=gstat[:, 0:1], in1=gstat[:, 0:1])
    nc.vector.tensor_sub(out=varg, in0=gstat[:, 1:2], in1=varg)
    # rstd = rsqrt(var + eps) via bit-trick seed + 2 Newton steps on DVE
    # (the ACT Sqrt lives in a different table set than Exp; using it would
    # put a ~1.3us ACT_TABLE_LOAD on the critical path twice)
    nc.vector.tensor_scalar_add(out=varg, in0=varg, scalar1=EPS)
    I32 = mybir.dt.int32
    magic = smallp.tile([C, 1], I32)
    nc.vector.memset(magic, 0x5F3759DF)
    rstd = smallp.tile([C, 1], F32)
    nc.vector.tensor_scalar(out=rstd.bitcast(I32), in0=varg.bitcast(I32),
                            scalar1=1, scalar2=None,
                            op0=mybir.AluOpType.arith_shift_right)
    nc.vector.tensor_sub(out=rstd.bitcast(I32), in0=magic, in1=rstd.bitcast(I32))
    nt = smallp.tile([C, 1], F32)
    for _ in range(2):
        nc.vector.tensor_mul(out=nt, in0=varg, in1=rstd)
        nc.vector.tensor_mul(out=nt, in0=nt, in1=rstd)
        nc.vector.tensor_scalar(out=nt, in0=nt, scalar1=-0.5, scalar2=1.5,
                                op0=mybir.AluOpType.mult,
                                op1=mybir.AluOpType.add)
        nc.vector.tensor_mul(out=rstd, in0=rstd, in1=nt)
    # xn = x*s1 + s2 with s1 = rstd*gamma, s2 = beta - mean*s1
    s1 = smallp.tile([C, 1], F32)
    s2 = smallp.tile([C, 1], F32)
    nc.vector.tensor_mul(out=s1, in0=rstd, in1=gamma)
    nc.vector.tensor_mul(out=s2, in0=gstat[:, 0:1], in1=s1)
    nc.vector.tensor_sub(out=s2, in0=beta, in1=s2)
    # minimal pre-attention work: xn/K/Q for the first two 512-slices only;
    # everything else streams in as gap-filler during chunk 0. Order matters:
    # engine queues are in-order, so nothing slow may sit ahead of the ops
    # that gate the first exp.
    q_bf = bigp.tile([C, HW], BF16)   # Q^T channel-major
    k_bf = bigp.tile([C, HW], BF16)   # K^T
    v_ext = bigp.tile([C, NKB, VE], BF16)  # V token-major + ones column
    nc.vector.memset(v_ext[:, :, 128:129], 1.0)
    xn_sb = bigp.tile([C, HW], mybir.dt.float32r, tag="xn_on", name="xn_sb")

    def xn_slice(h):
        sl = slice(h * 512, (h + 1) * 512)
        nc.vector.tensor_scalar(out=xn_sb[:, sl], in0=x_sb[:, sl],
                                scalar1=s1, scalar2=s2,
                                op0=mybir.AluOpType.mult, op1=mybir.AluOpType.add)

    def k_slice(h, pool, tag):
        sl = slice(h * 512, (h + 1) * 512)
        ps_k = pool.tile([C, 512], F32, tag=tag, name="ps_k")
        nc.tensor.matmul(ps_k[:, 0:512], wkT_r, xn_sb[:, sl])
        nc.vector.tensor_scalar_add(out=k_bf[:, sl], in0=ps_k[:, 0:512], scalar1=bk)

    def q_slice(h, pool, tag):
        sl = slice(h * 512, (h + 1) * 512)
        ps_q = pool.tile([C, 512], F32, tag=tag, name="ps_q")
        nc.tensor.matmul(ps_q[:, 0:512], wqT_r, xn_sb[:, sl])
        nc.vector.tensor_scalar_add(out=q_bf[:, sl], in0=ps_q[:, 0:512], scalar1=bq)

    xn_slice(0)
    xn_slice(1)
    k_slice(0, ps_s, "ps_s")
    k_slice(1, ps_s, "ps_s")
    q_slice(0, ps_t, "ps_t")
    q_slice(1, ps_t, "ps_t")

    def head_work():
        """Remaining xn/K/Q slices and all V blocks, emitted in unit-sized
        slices as PE gap-filler inside chunk 0's S/exp stream. Uses ps_t/ps_n
        pools so it never competes for the S-tile PSUM slots."""
        for h in range(2, HW // 512):
            xn_slice(h)
            k_slice(h, ps_t, "ps_t")
            q_slice(h, ps_t, "ps_t")
            yield
        for kb0 in range(0, NKB, 4):
            for kb in range(kb0, kb0 + 4):
                ps_v = ps_n.tile([C, VE], F32, tag="ps_n", name="ps_v")
                nc.tensor.matmul(ps_v[:, 0:KBLK],
                                 xn_sb[:, kb * KBLK:(kb + 1) * KBLK], wvT_r)
                nc.vector.tensor_add(out=v_ext[:, kb, 0:128],
                                     in0=ps_v[:, 0:KBLK], in1=bv_bc)
            yield

    # ---- attention ----
    on_sb = bigp.tile([C, HW], mybir.dt.float32r, tag="xn_on", name="on_sb")
    PV_SLICE = 8  # PV matmuls per S/exp unit (~0.7us of PE gap-filler)

    def proj_out(h):
        sl = slice(h * 512, (h + 1) * 512)
        ps_f = ps_t.tile([C, 512], F32, tag="ps_t", name="ps_f")
        nc.tensor.matmul(ps_f[:, 0:512], woT_r, on_sb[:, sl])
        ot = outp.tile([C, 512], F32, name="ot")
        nc.vector.scalar_tensor_tensor(out=ot, in0=ps_f[:, 0:512], scalar=bo,
                                       in1=x_sb[:, sl],
                                       op0=mybir.AluOpType.add,
                                       op1=mybir.AluOpType.add)
        nc.sync.dma_start(out=d["out"][:, sl], in_=ot)

    def tail_work(qc, pts):
        """PV + normalize + transpose + output projection for chunk qc,
        yielded in ~unit-sized slices to pace evenly against S/exp units."""
        for sub in range(NSUB):
            qpos = qc * QCHUNK + sub * KBLK
            pnat = ps_n.tile([C, VE], F32, tag="ps_n", name="pnat")
            for kb0 in range(0, NKB, PV_SLICE):
                for kb in range(kb0, kb0 + PV_SLICE):
                    nc.tensor.matmul(pnat[:, 0:129],
                                     pts[kb][:, sub * KBLK:(sub + 1) * KBLK],
                                     v_ext[:, kb, 0:129],
                                     start=(kb == 0), stop=(kb == NKB - 1))
                if kb0 + PV_SLICE < NKB:
                    yield
            rz = smallp.tile([C, 1], F32, name="rz")
            nc.vector.reciprocal(out=rz, in_=pnat[:, 128:129])
            onat = outp.tile([C, KBLK], F32, name="onat")
            nc.vector.tensor_scalar_mul(out=onat, in0=pnat[:, 0:128], scalar1=rz)
            ptr = ps_t.tile([C, KBLK], F32, tag="ps_t", name="ptr")
            nc.tensor.transpose(ptr, onat, ident)
            nc.vector.tensor_copy(out=on_sb[:, qpos:qpos + KBLK], in_=ptr)
            if sub % 4 == 3:
                proj_out(qc * 2 + sub // 4)
            yield

    filler = head_work()
    for qc in range(NQC):
        q0 = qc * QCHUNK
        pts = []
        for kb in range(NKB):
            ksl = slice(kb * KBLK, (kb + 1) * KBLK)
            ps_sc = ps_s.tile([C, QCHUNK], F32, tag="ps_s", name="ps_sc")
            nc.tensor.matmul(ps_sc[:, 0:512], k_bf[:, ksl], q_bf[:, q0:q0 + 512])
            nc.tensor.matmul(ps_sc[:, 512:1024], k_bf[:, ksl],
                             q_bf[:, q0 + 512:q0 + 1024])
            pt = ptp.tile([C, QCHUNK], BF16, name="pt")
            nc.scalar.activation(out=pt, in_=ps_sc,
                                 func=mybir.ActivationFunctionType.Exp,
                                 scale=scale)
            pts.append(pt)
            next(filler, None)
        filler = tail_work(qc, pts)
    for _ in filler:
        pass


_CACHE = {}


def _build():
    if "nc" in _CACHE:
        return _CACHE["nc"]
    nc = bacc.Bacc("TRN2", target_bir_lowering=False, debug=False,
                   num_devices=NCORES)
    d = {}
    d["x"] = nc.dram_tensor("x", [C, HW], F32, kind="ExternalInput").ap()
    d["cmat"] = nc.dram_tensor("cmat", [C, 7, C], F32, kind="ExternalInput").ap()
    d["cvec"] = nc.dram_tensor("cvec", [C, 5], F32, kind="ExternalInput").ap()
    d["out"] = nc.dram_tensor("out", [C, HW], F32, kind="ExternalOutput").ap()
    with tile.TileContext(nc) as tc:
        with ExitStack() as ctx:
            _emit(tc, d, ctx)
    nc.compile()
    _CACHE["nc"] = nc
    return nc


def make_in_maps(x, gamma, beta, wq, bq, wk, bk, wv, bv, wo, bo):
    f = np.float32
    gm = np.zeros((C, C), f)
    for g in range(GROUPS):
        gm[g * GSIZE:(g + 1) * GSIZE, g * GSIZE:(g + 1) * GSIZE] = 1.0 / GSIZE
    cmat = np.stack([
        np.asarray(wq, f).T, np.asarray(wk, f).T, np.asarray(wv, f).T,
        np.asarray(wo, f).T, gm,
        np.tile(np.asarray(bv, f).reshape(1, C), (C, 1)),
        np.eye(C, dtype=f),
    ], axis=1)  # [C, 7, C]
    cvec = np.stack([np.asarray(v, f).reshape(C) for v in
                     (bq, bk, bo, gamma, beta)], axis=1)  # [C, 5]
    common = {
        "cmat": np.ascontiguousarray(cmat),
        "cvec": np.ascontiguousarray(cvec),
    }
    xf = np.asarray(x, f).reshape(B, C, HW)
    return [dict(common, x=np.ascontiguousarray(xf[b])) for b in range(B)]


def kernel(x, gamma, beta, wq, bq, wk, bk, wv, bv, wo, bo, **run_kwargs):
    nc = _build()
    in_maps = make_in_maps(x, gamma, beta, wq, bq, wk, bk, wv, bv, wo, bo)
    res = run_bass_kernel_spmd(nc, in_maps, core_ids=list(range(NCORES)),
                               **run_kwargs)
    out = np.stack([res.results[b]["out"] for b in range(B)])
    _CACHE["last_results"] = res
    return out.reshape(B, C, 64, 64).astype(np.float32)
